# revision 18
# baseline (speedup 1.0000x reference)
"""CQC contrastive loss kernel for 8 Trainium2 NeuronCores.

Math (B=4096, D=256, TAU=0.5, N=2B=8192):
    x  = concat(Xa, Za)                      [N, D]
    xn = x / ||x||                           (row-normalized)
    S  = xn @ xn.T                           [N, N]
    loss_i = log(sum_{j != i} exp(S_ij/TAU)) - S[i, i+-B]/TAU
    loss   = mean_i loss_i

Split of work (wall time of a warm call is dominated by the axon tunnel:
tens-of-ms round trips, ~75 MB/s host->device, so the design minimizes
bytes moved and round trips, not device cycles):

  Host: row-normalize x in f32 (numpy) and cast to fp8 e4m3 via a small
      XLA-cpu jit (wire format only -- simulated end-to-end rel err
      1.2e-6), in 4 row chunks whose async sharded device_puts stream
      while the next chunk is computed; positive-pair dot sum
      pos_i = xn_i . xn_{i+-B} overlaps the upload tail. Only 2 MB
      crosses the tunnel; shard_map slices each chunk's axis 0 into the
      8 per-core pieces with no per-core host copies.
  Device (per core c, rows [1024c, 1024c+1024)): AllGather the 8 fp8 slabs
      over NeuronLink into the full xn [N, D] (rank order; the row-sum over
      all columns is permutation-invariant so gather order never matters),
      PE-transpose the fp8 tiles and cast to bf16 on the PSUM->SBUF copy,
      bf16 matmuls of the own-slab block against all N columns accumulating
      S in PSUM, ScalarE exp(2*S) with fused row-sum, then
      lg_i = log(rowsum_i - exp(2*||xn_i||^2)), reduce the 8 row blocks and
      DMA out [128, 1] per core.
  Host: loss = (sum_i lg_i - 2 * sum_i pos_i) / N.

The jitted executable, the Bass module, and the compiled NEFF are cached at
module level: warm calls pay only host math, the 2 MB upload, one execute
round trip, and one 4 KB fetch.
"""

import numpy as np
import ml_dtypes

import jax
import jax.numpy as jnp
from jax.sharding import Mesh, NamedSharding, PartitionSpec

try:
    from jax.experimental.shard_map import shard_map
except ImportError:  # newer jax
    from jax import shard_map

import concourse.bacc as bacc
import concourse.tile as tile
from concourse import mybir
from concourse import bass2jax

F32 = mybir.dt.float32
BF16 = mybir.dt.bfloat16
F8 = mybir.dt.float8e4
AL = mybir.AluOpType
AF = mybir.ActivationFunctionType

WIRE_NP = ml_dtypes.float8_e4m3
WIRE_JNP = jnp.float8_e4m3

B = 4096
D = 256
N = 2 * B
TAU = 0.5
NCORES = 8
RPC = N // NCORES          # rows per core = 1024
NBLK = RPC // 128          # 128-row blocks per core = 8
NCHUNK = 4                 # host->device upload pipeline chunks
CROWS = N // NCHUNK        # global rows per chunk = 2048
CPC = RPC // NCHUNK        # chunk rows per core = 256
NT = N // 128              # 128-row tiles in the gathered x = 64
GRP = 8                    # transpose phases (8 tiles each)
TPG = NT // GRP            # tiles per phase = 8
# main-loop chunk groups (in 512-col units): 16 chunks -> 6 groups sized to
# fit a 3-bank [128, 1536] f32 PSUM tile
CGS = [(0, 1, 2), (3, 4, 5), (6, 7, 8), (9, 10, 11), (12, 13, 14), (15,)]
NCG = len(CGS)


def _patch_act_tables():
    """Force every activation onto the one table set that covers both exp
    and ln, so the kernel pays a single ACT table load instead of two.
    Indices of the other sets are kept (emptied, not removed) because
    act_func_set_id is a positional index into act_info.json."""
    if getattr(bacc, "_cqc_act_patch", False):
        return
    orig = bacc.get_activation_tables

    def patched(module_arch):
        tabs = orig(module_arch)
        keep = "natural_log_exp_and_others"
        if keep in tabs:
            tabs = {name: (fns if name == keep else set())
                    for name, fns in tabs.items()}
        return tabs

    bacc.get_activation_tables = patched
    bacc._cqc_act_patch = True


def build():
    _patch_act_tables()
    nc = bacc.Bacc("TRN2", target_bir_lowering=False, debug=False,
                   num_devices=NCORES)

    Xcs = [nc.dram_tensor(f"Xc{k}", [CPC, D], F8, kind="ExternalInput").ap()
           for k in range(NCHUNK)]
    oLoss = nc.dram_tensor("loss", [128, 1], F32,
                           kind="ExternalOutput").ap()
    ident = nc.inline_tensor(np.eye(128, dtype=WIRE_NP), name="ident").ap()

    with tile.TileContext(nc) as tc:
        with (
            tc.tile_pool(name="dram", bufs=1, space="DRAM") as dr,
            tc.tile_pool(name="stream", bufs=3) as st,
            tc.tile_pool(name="persist", bufs=1) as pr,
            tc.tile_pool(name="psum", bufs=2, space="PSUM") as ps,
        ):
            # --- AllGather the normalized slabs (bounce via internal DRAM).
            # The slab arrives as NCHUNK pipelined upload chunks; their
            # concatenation (and hence the gathered row order) is a fixed
            # permutation of the global rows, which is harmless: the row-sum
            # runs over all columns and the host only consumes the SUM of
            # the per-row losses. ---
            inb = dr.tile([RPC, D], F8)
            for k in range(NCHUNK):
                nc.gpsimd.dma_start(inb[k * CPC:(k + 1) * CPC, :], Xcs[k])
            gx = dr.tile([N, D], F8, addr_space="Shared")
            nc.gpsimd.collective_compute(
                "AllGather", AL.bypass,
                replica_groups=[list(range(NCORES))],
                ins=[inb], outs=[gx])
            gxt = gx.rearrange("(t p) d -> p t d", p=128)   # [128, 64, 256]
            Xst = inb.rearrange("(t p) d -> p t d", p=128)  # [128, 8, 256]

            idt = pr.tile([128, 128], F8, tag="ident")
            nc.sync.dma_start(out=idt, in_=ident)

            sdiag = pr.tile([128, NBLK], F32, tag="sdiag")
            rs_parts = pr.tile([128, NBLK * NCG], F32, tag="rsp")

            # xnT[k][g]: [128, 1024] bf16 -- d-half k, 1024-col group g
            xnT = [[pr.tile([128, TPG * 128], BF16, tag=f"xnT{k}_{g}",
                            name=f"xnT{k}_{g}")
                    for g in range(GRP)] for k in range(2)]
            # lhsT[k]: [128, 1024] bf16 -- transposed own slab, block b at
            # cols [128b, 128b+128)
            lhsT = [pr.tile([128, RPC], BF16, tag=f"lhsT{k}",
                            name=f"lhsT{k}") for k in range(2)]

            def own_slab():
                xs = pr.tile([128, NBLK, D], F8, tag="xs")
                nc.sync.dma_start(out=xs, in_=Xst)
                for t in range(NBLK):
                    xb = st.tile([128, D], BF16, tag="xb", name="xb")
                    nc.vector.tensor_copy(xb, xs[:, t, :])
                    scr = st.tile([128, D], BF16, tag="sq", name="sq")
                    nc.vector.scalar_tensor_tensor(
                        out=scr, in0=xb, scalar=1.0, in1=xb,
                        op0=AL.mult, op1=AL.mult,
                        accum_out=sdiag[:, t:t + 1])
                for k in range(2):
                    # fp8 PE transpose requires output element step 2 in PSUM
                    pt = ps.tile([128, NBLK * 128, 2], F8, tag="tp", name="pt")
                    for t in range(NBLK):
                        nc.tensor.transpose(
                            pt[:, t * 128:(t + 1) * 128, 0],
                            xs[:, t, k * 128:(k + 1) * 128], idt)
                    nc.vector.tensor_copy(lhsT[k], pt[:, :, 0])

            def phase0(g):
                xg = st.tile([128, TPG, D], F8, tag="xg", name="xg")
                nc.sync.dma_start(out=xg, in_=gxt[:, g * TPG:(g + 1) * TPG, :])
                for k in range(2):
                    pt = ps.tile([128, TPG * 128, 2], F8, tag="tp", name="pt")
                    for t in range(TPG):
                        nc.tensor.transpose(
                            pt[:, t * 128:(t + 1) * 128, 0],
                            xg[:, t, k * 128:(k + 1) * 128], idt)
                    nc.vector.tensor_copy(xnT[k][g], pt[:, :, 0])

            def main_cg(b, cgi):
                cg = CGS[cgi]
                w = len(cg) * 512
                pm = ps.tile([128, w], F32, tag="big", name="pm",
                             padded_shape=[128, 3 * 512])
                for k in range(2):
                    lh = lhsT[k][:, b * 128:(b + 1) * 128]
                    for i, c in enumerate(cg):
                        nc.tensor.matmul(
                            pm[:, i * 512:(i + 1) * 512], lh,
                            xnT[k][c // 2]
                               [:, (c % 2) * 512:(c % 2 + 1) * 512],
                            start=(k == 0), stop=(k == 1))
                escr = st.tile([128, w], BF16, tag="exps", name="exps",
                               padded_shape=[128, 3 * 512])
                col = b * NCG + cgi
                nc.scalar.activation(
                    out=escr, in_=pm, func=AF.Exp, scale=2.0,
                    accum_out=rs_parts[:, col:col + 1])

            own_slab()
            for g in range(GRP):
                phase0(g)
            for b in range(NBLK):
                for cgi in range(NCG):
                    main_cg(b, cgi)

            # --- finals: lg = log(rowsum - exp(2*sdiag)), reduce blocks ---
            rs_tot = pr.tile([128, NBLK], F32, tag="rs_tot")
            nc.vector.tensor_reduce(
                out=rs_tot,
                in_=rs_parts.rearrange("p (b g) -> p b g", g=NCG),
                op=AL.add, axis=mybir.AxisListType.X)
            e_diag = pr.tile([128, NBLK], F32, tag="e_diag")
            nc.scalar.activation(out=e_diag, in_=sdiag, func=AF.Exp,
                                 scale=2.0)
            rsm = pr.tile([128, NBLK], F32, tag="rsm")
            nc.vector.tensor_sub(rsm, rs_tot, e_diag)
            lg = pr.tile([128, NBLK], F32, tag="lg")
            nc.scalar.activation(out=lg, in_=rsm, func=AF.Ln)
            lgs = pr.tile([128, 1], F32, tag="lgs")
            nc.vector.tensor_reduce(out=lgs, in_=lg, op=AL.add,
                                    axis=mybir.AxisListType.X)
            nc.sync.dma_start(out=oLoss, in_=lgs)

    nc.finalize()
    return nc


_CACHE = {}
last_results = None


@jax.jit
def _div_cast(X, nrm):
    # fused normalize + fp8 cast; runs on CPU (inputs are committed there)
    return (X / nrm[:, None]).astype(WIRE_JNP)


def _setup():
    nc = build()
    bass2jax.install_neuronx_cc_hook()

    partition_name = (nc.partition_id_tensor.name
                      if nc.partition_id_tensor else None)
    in_names, out_names, out_avals = [], [], []
    for alloc in nc.m.functions[0].allocations:
        if not isinstance(alloc, mybir.MemoryLocationSet):
            continue
        name = alloc.memorylocations[0].name
        if alloc.kind == "ExternalInput":
            if name != partition_name:
                in_names.append(name)
        elif alloc.kind == "ExternalOutput":
            out_names.append(name)
            out_avals.append(jax.core.ShapedArray(
                tuple(alloc.tensor_shape), mybir.dt.np(alloc.dtype)))
    assert in_names == [f"Xc{k}" for k in range(NCHUNK)], in_names
    assert out_names == ["loss"], out_names
    n_params = len(in_names)
    n_outs = len(out_avals)
    # No donated zero output buffers: the kernel writes every element of
    # "loss", and the neuronx hook renames it to output0 anyway (out_rename
    # wins the dict union), so a donated operand would bind to nothing.
    in_names_full = in_names + ([partition_name] if partition_name else [])

    def _body(*args):
        operands = list(args)
        if partition_name is not None:
            operands.append(bass2jax.partition_id_tensor())
        outs = bass2jax._bass_exec_p.bind(
            *operands, out_avals=tuple(out_avals),
            in_names=tuple(in_names_full), out_names=tuple(out_names),
            lowering_input_output_aliases=(),
            sim_require_finite=True, sim_require_nnan=True, nc=nc)
        return tuple(outs)

    devices = jax.devices()[:NCORES]
    assert len(devices) == NCORES, (
        f"need {NCORES} devices, found {len(jax.devices())}")
    mesh = Mesh(np.asarray(devices), ("core",))
    sharded = jax.jit(
        shard_map(_body, mesh=mesh,
                  in_specs=(PartitionSpec("core"),) * n_params,
                  out_specs=(PartitionSpec("core"),) * n_outs,
                  check_rep=False),
        keep_unused=True)
    _CACHE["fn"] = sharded
    _CACHE["sharding"] = NamedSharding(mesh, PartitionSpec("core"))


def kernel(Xa: np.ndarray, Za: np.ndarray) -> np.ndarray:
    if "fn" not in _CACHE:
        _setup()
    fn = _CACHE["fn"]
    sh = _CACHE["sharding"]
    cpu = jax.devices("cpu")[0]

    # --- host: normalize (f32) + fp8 cast (XLA cpu) per chunk; each chunk's
    # upload streams over the tunnel while the next chunk is computed ---
    Xa = np.asarray(Xa)
    Za = np.asarray(Za)
    X = np.empty((N, D), np.float32)
    nrm = np.empty((N,), np.float32)
    dchunks = []
    for k in range(NCHUNK):
        lo = k * CROWS
        src = Xa if lo < B else Za
        s0 = lo % B
        Xk = X[lo:lo + CROWS]
        Xk[:] = src[s0:s0 + CROWS]
        nk = np.maximum(np.sqrt(np.einsum("ij,ij->i", Xk, Xk)), 1e-8)
        nrm[lo:lo + CROWS] = nk
        qk = _div_cast(jax.device_put(Xk, cpu), jax.device_put(nk, cpu))
        dchunks.append(jax.device_put(qk, sh))   # async upload

    # pos on raw rows (overlaps the tail of the uploads):
    # pos_i = (x_i . x_{i+B}) / (|x_i| |x_{i+B}|)
    pd = np.einsum("ij,ij->i", X[:B], X[B:])
    p0sum = float((pd / (nrm[:B] * nrm[B:])).sum(dtype=np.float64))

    out = fn(*dchunks)                           # async dispatch to trn2
    lg = np.asarray(out[0])                      # [8*128, 1]

    loss = (lg.astype(np.float64).sum() - 4.0 * p0sum) / N
    return np.float32(loss)
